# revision 1
# baseline (speedup 1.0000x reference)
"""Trainium2 Bass kernel for C = triu(triu(A) @ triu(B)), N=4096, fp32.

Math: with host-side triu masking of A and B, the product is upper-triangular
automatically; for output element (r, c) only k in [r, c] contributes.

Sharding (8 cores, SPMD, one NEFF): a 4x2 grid.
  - Rows: 4 row-groups, cyclic mod 4 at 128-row tile granularity. Core with
    row-group r owns row-tiles {4t + r : t = 0..7} (8 slots of 128 rows).
  - Columns: 2 column-groups by n-tile parity (h = 0 even, h = 1 odd
    128-column tiles). A core owns 16 n-tiles {2u + h}, grouped into 4
    "virtual supers" v = 0..3 of 4 owned tiles {8v + 2j + h : j = 0..3}
    (512 packed columns each).
Interleaving parities keeps the SPMD loop bounds nearly tight for both
column-groups: vsuper v needs k-tiles k <= 8v + 6 + h, the program runs the
union k <= 8v+7. Where a core's data has no work the packed operands are
zero, so the extra matmuls accumulate zeros and stay correct.

Per-core traffic (bf16): B ragged-trimmed 8.9 MB, A triu-trimmed 4.7 MB,
C out 2.4 MB (bf16, host converts back to fp32) ~= 16 MB, balancing the
~888 128^3-tile-matmul units of tensor work per core.

Schedule notes:
  - ~20 warmup matmuls on memset-zero SBUF run during the NEFF preamble /
    first-DMA window so the PE HAM clock-gate reaches 8/8 before real work.
  - vsupers processed in order 1,2,3,0: best early compute-per-DMA-byte and
    the 8-copy drain of v3 overlaps v0 instead of being a tail.
  - PSUM->SBUF copies alternate VectorE / ScalarE (ACT); the A-shard chunk
    loads are emitted between copies on the Scalar queue, so they dispatch
    just-in-time instead of competing with early B streaming.
  - C stores are batched per vsuper halves (2 GpSimd DMAs per vsuper).
"""

import sys

for _p in ("/opt/trn_rl_repo", "/root/.axon_site/_ro/trn_rl_repo"):
    if _p not in sys.path:
        sys.path.insert(0, _p)

import numpy as np

N = 4096
P = 128
NCORES = 8
NSLOT = 8  # row-tiles per core (cyclic mod 4)
NV = 4  # virtual supers per core
SW = 512  # packed columns per vsuper
KT = N // P  # 32 k-tiles
N_WARM = 10  # warmup matmuls (512 wide) to flip the HAM clock gate early
VORDER = [1, 2, 3, 0]
# B k-chunk boundaries per vsuper (~1MB DMAs; v1's split finer for startup)
BCHUNKS = {1: [0, 4, 8, 16], 2: [0, 8, 16, 24], 3: [0, 8, 16, 24, 32], 0: [0, 8]}
BMAXK = 8

_cache = {}


def _kmax(v):
    return 8 * v + 7


def _j0(k, v):
    # first owned-n-tile index j (0..3) inside vsuper v that can still have
    # a nonzero triu(B) entry at k-tile k (taking the wider h=1 parity)
    return max(0, (k - 8 * v) // 2)


def _wtiles(k, v):
    return 4 - _j0(k, v)


# --- A pack layout: k-major, slots t <= k//4, trimmed to k >= 4t ---
def _aoff(k):
    # column offset (in elements) of k-tile k's slot block in the A pack
    return 128 * sum(kk // 4 + 1 for kk in range(k))


A_COLS = _aoff(KT)  # 144 * 128 = 18432


# --- B pack layout: per vsuper, per k, ragged width (4 - j0) * 128 ---
def _boff(k, v):
    return 128 * sum(_wtiles(kk, v) for kk in range(k))


_BBASE = []
_b = 0
for _v in range(NV):
    _BBASE.append(_b)
    _b += _boff(_kmax(_v) + 1, _v)
B_COLS = _b  # 272 * 128 = 34816


# --- C pack layout: blocks (v, t) for t <= 2v+1, width (4 - j0(4t, v)) * 128
def _cwidth(v, t):
    return 128 * (4 - _j0(4 * t, v))


_CBASE = {}
_c = 0
for _v in range(NV):
    for _t in range(2 * _v + 2):
        _CBASE[(_v, _t)] = _c
        _c += _cwidth(_v, _t)
C_COLS = _c  # 9216
OT_COLS = max(
    _CBASE[(_v, 2 * _v + 1)] + _cwidth(_v, 2 * _v + 1) - _CBASE[(_v, 0)]
    for _v in range(NV)
)  # 3840


def _build():
    import concourse.bacc as bacc
    import concourse.mybir as mybir
    import concourse.tile as tile

    D = mybir.dt.bfloat16
    Copy = mybir.ActivationFunctionType.Copy

    nc = bacc.Bacc(None, target_bir_lowering=False)
    AT = nc.dram_tensor("AT", [P, A_COLS], D, kind="ExternalInput")
    Bm = nc.dram_tensor("B", [P, B_COLS], D, kind="ExternalInput")
    Cm = nc.dram_tensor("C", [P, C_COLS], D, kind="ExternalOutput")

    with tile.TileContext(nc) as tc:
        with (
            tc.tile_pool(name="w", bufs=1) as wpool,
            tc.tile_pool(name="a", bufs=4) as apool,
            tc.tile_pool(name="b", bufs=5) as bpool,
            tc.tile_pool(name="o", bufs=2) as opool,
            tc.tile_pool(name="ps", bufs=8, space="PSUM") as pspool,
        ):
            # --- PE warmup: flip HAM to 8/8 during preamble + first DMAs ---
            warm = wpool.tile([P, SW], D, tag="wm", name="warm")
            nc.vector.memset(warm[:], 0)
            wps = pspool.tile([P, SW], mybir.dt.float32, tag="ps", name="ps")
            for _ in range(N_WARM):
                nc.tensor.matmul(
                    wps[:], warm[:, :P], warm[:], start=True, stop=True
                )

            # --- A shard: 4 chunks by k-group; g0/g1 up front, g2/g3 JIT ---
            a_tiles = [None] * 4
            a_starts = [_aoff(8 * g) for g in range(5)]

            def load_a(g):
                ag = apool.tile(
                    [P, a_starts[g + 1] - a_starts[g]], D, tag=f"a{g}", name="ag"
                )
                nc.scalar.dma_start(ag[:], AT[:, a_starts[g] : a_starts[g + 1]])
                a_tiles[g] = ag

            load_a(0)
            # hold the later A chunks out of the congested early window
            # (g1/g2/g3 data is first used at ~13us / ~28us / ~42us)
            with tc.tile_wait_until(0.007):
                load_a(1)
            with tc.tile_wait_until(0.014):
                load_a(2)
            with tc.tile_wait_until(0.020):
                load_a(3)

            def a_sl(k, t):
                g = k // 8
                c0 = _aoff(k) - a_starts[g] + 128 * t
                return a_tiles[g][:, c0 : c0 + 128]

            for vi, v in enumerate(VORDER):
                kmax = _kmax(v)
                nslots = 2 * v + 2
                psums = [
                    pspool.tile([P, SW], mybir.dt.float32, tag="ps", name="ps")
                    for _ in range(nslots)
                ]
                bb = BCHUNKS[v]
                for ci, (kc, kend) in enumerate(zip(bb, bb[1:])):
                    cnt = kend - kc
                    c0 = _BBASE[v] + _boff(kc, v)
                    c1 = _BBASE[v] + _boff(kc + cnt, v)
                    bt = bpool.tile([P, BMAXK * SW], D, tag="b", name="bt")
                    nc.sync.dma_start(bt[:, : c1 - c0], Bm[:, c0:c1])
                    for k in range(kc, kc + cnt):
                        w0 = 128 * _j0(k, v)
                        b0 = _boff(k, v) - _boff(kc, v)
                        bw = 128 * _wtiles(k, v)
                        for t in range(k // 4 + 1):
                            nc.tensor.matmul(
                                psums[t][:, w0:SW],
                                a_sl(k, t),
                                bt[:, b0 : b0 + bw],
                                start=(k == 4 * t),
                                stop=(k == kmax),
                            )
                # drain PSUM -> SBUF (bf16), alternating Vector/Scalar, then
                # store in two batched DMAs on the GpSimd queue
                ot = opool.tile([P, OT_COLS], D, tag="o", name="ot")
                base = _CBASE[(v, 0)]
                for t in range(nslots):
                    w0 = 128 * _j0(4 * t, v)
                    cw = _cwidth(v, t)
                    l0 = _CBASE[(v, t)] - base
                    if t % 2 == 0:
                        nc.vector.tensor_copy(
                            ot[:, l0 : l0 + cw], psums[t][:, w0:SW]
                        )
                    else:
                        nc.scalar.activation(
                            ot[:, l0 : l0 + cw], psums[t][:, w0:SW], Copy
                        )
                half = nslots // 2
                lmid = _CBASE[(v, half)] - base
                lend = _CBASE[(v, nslots - 1)] - base + _cwidth(v, nslots - 1)
                # early vsupers' stores wait out the congested mid-kernel
                # window (B + late-A streaming); later ones go immediately
                cwait = {0: 0.042, 1: 0.050}.get(vi)
                ceng = nc.sync if vi == 2 else nc.gpsimd
                with tc.tile_wait_until(cwait or 0, enable=cwait is not None):
                    ceng.dma_start(Cm[:, base : base + lmid], ot[:, :lmid])
                    ceng.dma_start(
                        Cm[:, base + lmid : base + lend], ot[:, lmid:lend]
                    )
    nc.compile()
    return nc


def _get_nc():
    if "nc" not in _cache:
        _cache["nc"] = _build()
    return _cache["nc"]


def _make_in_maps(A, B):
    import ml_dtypes

    bf16 = np.dtype(ml_dtypes.bfloat16)
    A = np.asarray(A, dtype=np.float32)
    B = np.asarray(B, dtype=np.float32)
    Au = np.triu(A).astype(bf16)
    Bu = np.triu(B).astype(bf16)

    # A packs per row-group r: [p, k-major slots]
    a_packs = []
    for r in range(4):
        ATr = np.zeros((P, A_COLS), dtype=bf16)
        for k in range(KT):
            base = _aoff(k)
            for t in range(k // 4 + 1):
                m = 4 * t + r
                # lhsT[p, ml] = Au[128*m + ml, 128*k + p]
                ATr[:, base + 128 * t : base + 128 * (t + 1)] = Au[
                    128 * m : 128 * m + 128, 128 * k : 128 * k + 128
                ].T
        a_packs.append(ATr)

    # B packs per column parity h
    b_packs = []
    for h in range(2):
        Bh = np.zeros((P, B_COLS), dtype=bf16)
        for v in range(NV):
            for k in range(_kmax(v) + 1):
                base = _BBASE[v] + _boff(k, v)
                for i, j in enumerate(range(_j0(k, v), 4)):
                    n = 8 * v + 2 * j + h
                    Bh[:, base + 128 * i : base + 128 * (i + 1)] = Bu[
                        128 * k : 128 * k + 128, 128 * n : 128 * n + 128
                    ]
        b_packs.append(Bh)

    in_maps = []
    for j in range(NCORES):
        r, h = j % 4, j // 4
        in_maps.append({"AT": a_packs[r], "B": b_packs[h]})
    return in_maps


def kernel(A, B):
    from concourse.bass_utils import run_bass_kernel_spmd

    in_maps = _make_in_maps(A, B)
    nc = _get_nc()
    res = run_bass_kernel_spmd(nc, in_maps, core_ids=list(range(NCORES)))

    C = np.zeros((N, N), dtype=np.float32)
    for jcore in range(NCORES):
        r, h = jcore % 4, jcore // 4
        Cj = np.asarray(res.results[jcore]["C"]).astype(np.float32)
        for v in range(NV):
            for t in range(2 * v + 2):
                m = 4 * t + r
                cb = _CBASE[(v, t)]
                for i, j in enumerate(range(_j0(4 * t, v), 4)):
                    n = 8 * v + 2 * j + h
                    C[128 * m : 128 * m + 128, 128 * n : 128 * n + 128] = Cj[
                        :, cb + 128 * i : cb + 128 * (i + 1)
                    ]
    return C



# revision 9
# speedup vs baseline: 1.1185x; 1.1185x over previous
"""Trainium2 Bass kernel for C = triu(triu(A) @ triu(B)), N=4096, fp32.

Math: with host-side triu masking of A and B, the product is upper-triangular
automatically; for output element (r, c) only k in [r, c] contributes.

Sharding (8 cores, SPMD, one NEFF): a 4x2 grid.
  - Rows: 4 row-groups, cyclic mod 4 at 128-row tile granularity. Core with
    row-group r owns row-tiles {4t + r : t = 0..7}.
  - Columns: 2 column-groups by 64-wide strip parity: core with parity h owns
    64-col strips {2u + h}. A vsuper v = 0..3 packs the core's 8 strips
    {16v + 2j + h : j = 0..7} into 512 contiguous columns. At 64-strip
    granularity both parities have IDENTICAL sparsity structure:
    strip j is live at k-tile k iff j >= j0(k,v) = max(0, k - 8v), and output
    slot t needs strips j >= jc0(v,t) = max(0, 4t - 8v) -- no SPMD union waste.

Per-core: 109056 matmul rows (45.4us at 2.4 GHz) and ~13.4 MB A+B in +
2.36 MB C out (bf16).

Schedule: A + B are fully SBUF-resident; every input DMA is issued eagerly at
program start on the sync queue in exact first-need order, so HBM streams at
full rate with no buffer backpressure. Work order puts high-slot-count k first
so the PE is never starved waiting on low-work-per-byte tiles:
  P1: v3 k=[8..27, 0..7, 28..31]   (8 psum banks)
  P2: v2 + v0, k=[8..23, 0..7(+v0)] (6+2 banks)
  P3: v1 k=[8..15, 0..7]            (4 banks; tail = 2 copies + 0.25MB store)
Each phase's k=0..7 block runs mid/late-phase so slot t0/t1 drains free their
PSUM banks before the next phase's first allocations bind them (ring order is
arranged so first-needed psums bind earliest-freed banks). Warmup matmuls on
zeroed SBUF ramp the PE clock during the NEFF preamble. PSUM->SBUF drains
alternate Vector/Scalar; C stores go on the gpsimd queue.
"""

import sys

for _p in ("/opt/trn_rl_repo", "/root/.axon_site/_ro/trn_rl_repo"):
    if _p not in sys.path:
        sys.path.insert(0, _p)

import numpy as np

N = 4096
P = 128
NCORES = 8
KT = N // P  # 32 k-tiles
SW = 512  # psum width (8 strips of 64)
N_WARM = 4

NSLOTS = {0: 2, 1: 4, 2: 6, 3: 8}
KMAX = {v: 8 * v + 7 for v in range(4)}


def _j0(k, v):
    return max(0, k - 8 * v)


def _wB(k, v):  # B block width (cols) at (v, k)
    return 64 * (8 - _j0(k, v))


def _jc0(v, t):
    return max(0, 4 * t - 8 * v)


def _wC(v, t):  # C block width (cols) for slot (v, t)
    return 64 * (8 - _jc0(v, t))


# --- A pack: k-major, slots t <= k//4, lhsT tiles [p=k-rows, 128 m-cols] ---
def _aoff(k):
    return 128 * sum(kk // 4 + 1 for kk in range(k))


A_COLS = _aoff(KT)  # 18432

# A chunks in DMA issue order: aq2..aq6, aq01, aq7 (quads of 4 k-tiles)
A_CHUNKS = {
    "aq2": (_aoff(8), _aoff(12)),
    "aq3": (_aoff(12), _aoff(16)),
    "aq4": (_aoff(16), _aoff(20)),
    "aq5": (_aoff(20), _aoff(24)),
    "aq6": (_aoff(24), _aoff(28)),
    "aq01": (_aoff(0), _aoff(8)),
    "aq7": (_aoff(28), _aoff(32)),
}

# --- execution k-orders per phase: high-slot-count k first, then k=0..7.
# Slots t>=2 only run in the hi part (4t >= 8), so they stop and drain at
# hi-part end, overlapping the lo part; t0/t1 (and v0) stop at phase end.
P1_KS = list(range(8, 32)) + list(range(0, 8))
P2_KS = list(range(8, 24)) + list(range(0, 8))
P3_KS = list(range(8, 16)) + list(range(0, 8))

# --- B chunk stream: (name, [(v, k), ...]) in exact consumption order ---
def _b_chunks():
    ch = []
    ch.append(("b3_a", [(3, k) for k in range(8, 12)]))
    ch.append(("b3_b", [(3, k) for k in range(12, 16)]))
    ch.append(("b3_c", [(3, k) for k in range(16, 20)]))
    ch.append(("b3_d", [(3, k) for k in range(20, 24)]))
    ch.append(("b3_e", [(3, k) for k in range(24, 28)]))
    ch.append(("b3_f", [(3, k) for k in range(28, 32)]))
    ch.append(("b3_lo", [(3, k) for k in range(0, 8)]))
    ch.append(("b2_a", [(2, k) for k in range(8, 16)]))
    ch.append(("b2_b", [(2, k) for k in range(16, 24)]))
    blo = []
    for k in range(8):
        blo.append((2, k))
        blo.append((0, k))
    ch.append(("b20_lo", blo))
    ch.append(("b1_a", [(1, k) for k in range(8, 16)]))
    ch.append(("b1_lo", [(1, k) for k in range(0, 8)]))
    return ch


B_CHUNKS = _b_chunks()

# chunk column layout within the single B DRAM tensor
B_CHUNK_BASE = {}
B_BLOCK_OFF = {}  # (v,k) -> (chunk_name, col offset within chunk, width)
_b = 0
for _name, _blocks in B_CHUNKS:
    B_CHUNK_BASE[_name] = _b
    _o = 0
    for _v, _k in _blocks:
        B_BLOCK_OFF[(_v, _k)] = (_name, _o, _wB(_k, _v))
        _o += _wB(_k, _v)
    _b += _o
B_COLS = _b  # 33792

# DMA issue order on the sync queue (A chunks interleaved at first need)
STREAM = [
    ("A", "aq2"),
    ("B", "b3_a"),
    ("A", "aq3"),
    ("B", "b3_b"),
    ("A", "aq4"),
    ("B", "b3_c"),
    ("A", "aq5"),
    ("B", "b3_d"),
    ("A", "aq6"),
    ("B", "b3_e"),
    ("A", "aq7"),
    ("B", "b3_f"),
    ("A", "aq01"),
    ("B", "b3_lo"),
    ("B", "b2_a"),
    ("B", "b2_b"),
    ("B", "b20_lo"),
    ("B", "b1_a"),
    ("B", "b1_lo"),
]

# --- C layout: blocks in drain-batch order ---
CBLOCKS = (
    [(3, t) for t in range(2, 8)]
    + [(3, 0), (3, 1)]
    + [(2, 0), (2, 1), (0, 0), (0, 1)]
    + [(2, t) for t in range(2, 6)]
    + [(1, 2), (1, 3)]
    + [(1, 0), (1, 1)]
)
CBASE = {}
_c = 0
for _v, _t in CBLOCKS:
    CBASE[(_v, _t)] = _c
    _c += _wC(_v, _t)
C_COLS = _c  # 9216

# drain batches: (name, [(v, t)...], n_store_dmas)
DRAIN_BATCHES = [
    ("c3hi", [(3, t) for t in range(2, 8)], 2),
    ("c3lo", [(3, 0), (3, 1)], 1),
    ("c20lo", [(2, 0), (2, 1), (0, 0), (0, 1)], 1),
    ("c2hi", [(2, t) for t in range(2, 6)], 1),
    ("c1hi", [(1, 2), (1, 3)], 1),
    ("c1lo", [(1, 0), (1, 1)], 1),
]

_cache = {}


def _build():
    import concourse.bacc as bacc
    import concourse.mybir as mybir
    import concourse.tile as tile

    D = mybir.dt.bfloat16
    F32 = mybir.dt.float32
    Copy = mybir.ActivationFunctionType.Copy

    nc = bacc.Bacc(None, target_bir_lowering=False)
    AT = nc.dram_tensor("AT", [P, A_COLS], D, kind="ExternalInput")
    Bm = nc.dram_tensor("B", [P, B_COLS], D, kind="ExternalInput")
    Cm = nc.dram_tensor("C", [P, C_COLS], D, kind="ExternalOutput")

    with tile.TileContext(nc) as tc:
        with (
            tc.tile_pool(name="w", bufs=1) as wpool,
            tc.tile_pool(name="a", bufs=1) as apool,
            tc.tile_pool(name="b", bufs=1) as bpool,
            tc.tile_pool(name="o", bufs=1) as opool,
            tc.tile_pool(name="ps", bufs=8, space="PSUM") as pspool,
        ):
            # --- PE warmup during preamble / first DMA window ---
            warm = wpool.tile([P, SW], D, tag="wm", name="warm")
            nc.vector.memset(warm[:], 0)
            wps = pspool.tile([P, SW], F32, tag="ps", name="ps")
            for _ in range(N_WARM):
                nc.tensor.matmul(
                    wps[:], warm[:, :P], warm[:], start=True, stop=True
                )

            # --- eager input DMAs, one queue, exact need order ---
            a_tiles = {}
            b_tiles = {}
            b_chunk_map = dict(B_CHUNKS)
            for kind, name in STREAM:
                if kind == "A":
                    c0, c1 = A_CHUNKS[name]
                    at = apool.tile([P, c1 - c0], D, tag=name, name="at")
                    nc.sync.dma_start(at[:], AT[:, c0:c1])
                    a_tiles[name] = (at, c0)
                else:
                    base = B_CHUNK_BASE[name]
                    w = sum(_wB(k, v) for v, k in b_chunk_map[name])
                    bt = bpool.tile([P, w], D, tag=name, name="bt")
                    nc.sync.dma_start(bt[:], Bm[:, base : base + w])
                    b_tiles[name] = bt

            def a_sl(k, t):
                q = k // 4
                name = "aq01" if q < 2 else f"aq{q}"
                at, c0 = a_tiles[name]
                off = _aoff(k) - c0 + 128 * t
                return at[:, off : off + 128]

            def b_sl(v, k):
                name, off, w = B_BLOCK_OFF[(v, k)]
                return b_tiles[name][:, off : off + w]

            # --- psum allocation helper (ring order is load-bearing) ---
            def alloc_ps():
                return pspool.tile([P, SW], F32, tag="ps", name="ps")

            def mm(ps, v, k, t, first_k, last_k):
                w0 = 64 * _j0(k, v)
                nc.tensor.matmul(
                    ps[:, w0:SW],
                    a_sl(k, t),
                    b_sl(v, k),
                    start=(k == first_k),
                    stop=(k == last_k),
                )

            drain_eng = [0]  # alternate vector/scalar

            def drain_and_store(batch_name, blocks, psums, n_dmas):
                cols = sum(_wC(v, t) for v, t in blocks)
                ot = opool.tile([P, cols], D, tag=batch_name, name="ot")
                l0 = 0
                for v, t in blocks:
                    w0 = 64 * _jc0(v, t)
                    cw = _wC(v, t)
                    src = psums[(v, t)][:, w0:SW]
                    if drain_eng[0] % 2 == 0:
                        nc.vector.tensor_copy(ot[:, l0 : l0 + cw], src)
                    else:
                        nc.scalar.activation(ot[:, l0 : l0 + cw], src, Copy)
                    drain_eng[0] += 1
                    l0 += cw
                base = CBASE[blocks[0]]
                splits = [cols * i // n_dmas for i in range(n_dmas + 1)]
                for s0, s1 in zip(splits, splits[1:]):
                    nc.gpsimd.dma_start(
                        Cm[:, base + s0 : base + s1], ot[:, s0:s1]
                    )

            # ============ P1: v3, k = [8..31, 0..7] ============
            ps3 = {}
            for t in range(8):
                ps3[(3, t)] = alloc_ps()
            for k in P1_KS:
                for t in range(min(k // 4, 7) + 1):
                    first_k = 4 * t if 4 * t >= 8 else 8
                    last_k = 7 if t <= 1 else 31
                    mm(ps3[(3, t)], 3, k, t, first_k, last_k)
                if k == 31:  # t2..t7 complete: drain during the lo block
                    drain_and_store("c3hi", [(3, t) for t in range(2, 8)], ps3, 2)
            drain_and_store("c3lo", [(3, 0), (3, 1)], ps3, 1)

            # ============ P2: v2 + v0, k = [8..23, 0..7] ============
            ps2 = {}
            # alloc order binds ring bufs: first-needed psums get
            # earliest-freed banks (v3 t0/t1 freed mid-P1, t2.. at k31).
            ps2[(2, 0)] = alloc_ps()  # buf of v3 t0 (freed mid-P1)
            ps2[(2, 1)] = alloc_ps()  # v3 t1 (freed mid-P1)
            ps2[(2, 2)] = alloc_ps()  # v3 t2 (freed at P1 end)
            ps2[(2, 3)] = alloc_ps()
            ps2[(2, 4)] = alloc_ps()
            ps2[(2, 5)] = alloc_ps()
            ps2[(0, 0)] = alloc_ps()
            ps2[(0, 1)] = alloc_ps()
            for k in P2_KS:
                for t in range(min(k // 4, 5) + 1):
                    first_k = 4 * t if 4 * t >= 8 else 8
                    last_k = 7 if t <= 1 else 23
                    mm(ps2[(2, t)], 2, k, t, first_k, last_k)
                if k < 8:  # lo block: v0 runs alongside
                    for t in range(min(k // 4, 1) + 1):
                        mm(ps2[(0, t)], 0, k, t, 4 * t, 7)
                if k == 23:  # t2..t5 complete: drain during the lo block
                    drain_and_store("c2hi", [(2, t) for t in range(2, 6)], ps2, 1)
            drain_and_store("c20lo", [(2, 0), (2, 1), (0, 0), (0, 1)], ps2, 1)

            # ============ P3: v1, k = [8..15, 0..7] ============
            ps1 = {}
            for t in range(4):
                ps1[(1, t)] = alloc_ps()
            for k in P3_KS:
                for t in range(min(k // 4, 3) + 1):
                    first_k = 4 * t if 4 * t >= 8 else 8
                    last_k = 7 if t <= 1 else 15
                    mm(ps1[(1, t)], 1, k, t, first_k, last_k)
                if k == 15:  # t2, t3 complete: drain during the lo block
                    drain_and_store("c1hi", [(1, 2), (1, 3)], ps1, 1)
            drain_and_store("c1lo", [(1, 0), (1, 1)], ps1, 1)

    nc.compile()
    return nc


def _get_nc():
    if "nc" not in _cache:
        _cache["nc"] = _build()
    return _cache["nc"]


def _make_in_maps(A, B):
    import ml_dtypes

    bf16 = np.dtype(ml_dtypes.bfloat16)
    A = np.asarray(A, dtype=np.float32)
    B = np.asarray(B, dtype=np.float32)
    Au = np.triu(A).astype(bf16)
    Bu = np.triu(B).astype(bf16)

    # A packs per row-group r: [p, k-major slots], lhsT layout
    a_packs = []
    for r in range(4):
        ATr = np.zeros((P, A_COLS), dtype=bf16)
        for k in range(KT):
            base = _aoff(k)
            for t in range(k // 4 + 1):
                m = 4 * t + r
                ATr[:, base + 128 * t : base + 128 * (t + 1)] = Au[
                    128 * m : 128 * m + 128, 128 * k : 128 * k + 128
                ].T
        a_packs.append(ATr)

    # B packs per column parity h: chunk-stream layout, 64-wide strips
    b_packs = []
    for h in range(2):
        Bh = np.zeros((P, B_COLS), dtype=bf16)
        for name, blocks in B_CHUNKS:
            base = B_CHUNK_BASE[name]
            for v, k in blocks:
                _, off, w = B_BLOCK_OFF[(v, k)]
                for i, j in enumerate(range(_j0(k, v), 8)):
                    n64 = 16 * v + 2 * j + h
                    Bh[:, base + off + 64 * i : base + off + 64 * (i + 1)] = (
                        Bu[128 * k : 128 * k + 128, 64 * n64 : 64 * n64 + 64]
                    )
        b_packs.append(Bh)

    in_maps = []
    for jcore in range(NCORES):
        r, h = jcore % 4, jcore // 4
        in_maps.append({"AT": a_packs[r], "B": b_packs[h]})
    return in_maps


def kernel(A, B):
    from concourse.bass_utils import run_bass_kernel_spmd

    in_maps = _make_in_maps(A, B)
    nc = _get_nc()
    res = run_bass_kernel_spmd(nc, in_maps, core_ids=list(range(NCORES)))

    C = np.zeros((N, N), dtype=np.float32)
    for jcore in range(NCORES):
        r, h = jcore % 4, jcore // 4
        Cj = np.asarray(res.results[jcore]["C"]).astype(np.float32)
        for v, t in CBLOCKS:
            m = 4 * t + r
            cb = CBASE[(v, t)]
            for i, j in enumerate(range(_jc0(v, t), 8)):
                n64 = 16 * v + 2 * j + h
                C[128 * m : 128 * m + 128, 64 * n64 : 64 * n64 + 64] = Cj[
                    :, cb + 64 * i : cb + 64 * (i + 1)
                ]
    return C


# revision 16
# speedup vs baseline: 1.1281x; 1.0086x over previous
"""Trainium2 Bass kernel for C = triu(triu(A) @ triu(B)), N=4096, fp32.

Math: with host-side triu masking of A and B, the product is upper-triangular
automatically; for output element (r, c) only k in [r, c] contributes.

Sharding (8 cores, SPMD, one NEFF): a 4x2 grid.
  - Rows: 4 row-groups, cyclic mod 4 at 128-row tile granularity. Core with
    row-group r owns row-tiles {4t + r : t = 0..7}.
  - Columns: 2 column-groups by 64-wide strip parity: core with parity h owns
    64-col strips {2u + h}. A vsuper v = 0..3 packs the core's 8 strips
    {16v + 2j + h : j = 0..7} into 512 contiguous columns. At 64-strip
    granularity both parities have IDENTICAL sparsity structure:
    strip j is live at k-tile k iff j >= j0(k,v) = max(0, k - 8v), and output
    slot t needs strips j >= jc0(v,t) = max(0, 4t - 8v) -- no SPMD union waste.

Per-core: 109056 matmul rows (45.4us at 2.4 GHz) and ~13.4 MB A+B in +
2.36 MB C out (bf16).

Schedule: A + B are fully SBUF-resident; every input DMA is issued eagerly at
program start on the sync queue in exact first-need order, so HBM streams at
full rate with no buffer backpressure. Work order puts high-slot-count k first
so the PE is never starved waiting on low-work-per-byte tiles:
  P1: v3 k=[8..27, 0..7, 28..31]   (8 psum banks)
  P2: v2 + v0, k=[8..23, 0..7(+v0)] (6+2 banks)
  P3: v1 k=[8..15, 0..7]            (4 banks; tail = 2 copies + 0.25MB store)
Each phase's k=0..7 block runs mid/late-phase so slot t0/t1 drains free their
PSUM banks before the next phase's first allocations bind them (ring order is
arranged so first-needed psums bind earliest-freed banks). Warmup matmuls on
zeroed SBUF ramp the PE clock during the NEFF preamble. PSUM->SBUF drains
alternate Vector/Scalar; C stores go on the gpsimd queue.
"""

import sys

for _p in ("/opt/trn_rl_repo", "/root/.axon_site/_ro/trn_rl_repo"):
    if _p not in sys.path:
        sys.path.insert(0, _p)

import numpy as np

N = 4096
P = 128
NCORES = 8
KT = N // P  # 32 k-tiles
SW = 512  # psum width (8 strips of 64)
N_WARM = 8  # keeps the PE continuously busy from preamble end (~7.9us) until
# the first B/A chunks have landed (~11.3us), so the HAM clock gate and
# p-state ramp complete without a reset

NSLOTS = {0: 2, 1: 4, 2: 6, 3: 8}
KMAX = {v: 8 * v + 7 for v in range(4)}


def _j0(k, v):
    return max(0, k - 8 * v)


def _wB(k, v):  # B block width (cols) at (v, k)
    return 64 * (8 - _j0(k, v))


def _jc0(v, t):
    return max(0, 4 * t - 8 * v)


def _wC(v, t):  # C block width (cols) for slot (v, t)
    return 64 * (8 - _jc0(v, t))


# --- A pack: k-major, slots t <= k//4, lhsT tiles [p=k-rows, 128 m-cols] ---
def _aoff(k):
    return 128 * sum(kk // 4 + 1 for kk in range(k))


A_COLS = _aoff(KT)  # 18432

# A chunks in DMA issue order (first quad split for startup latency)
A_CHUNKS = {
    "aq2a": (_aoff(8), _aoff(10)),
    "aq2b": (_aoff(10), _aoff(12)),
    "aq3": (_aoff(12), _aoff(16)),
    "aq4": (_aoff(16), _aoff(20)),
    "aq5": (_aoff(20), _aoff(24)),
    "aq6": (_aoff(24), _aoff(28)),
    "aq01": (_aoff(0), _aoff(8)),
    "aq7": (_aoff(28), _aoff(32)),
}


def _a_chunk_name(k):
    q = k // 4
    if q < 2:
        return "aq01"
    if q == 2:
        return "aq2a" if k < 10 else "aq2b"
    return f"aq{q}"

# --- execution k-orders per phase: high-slot-count k first, then k=0..7.
# Slots t>=2 only run in the hi part (4t >= 8), so they stop and drain at
# hi-part end, overlapping the lo part; t0/t1 (and v0) stop at phase end.
P1_KS = list(range(8, 32)) + list(range(0, 8))
P2_KS = list(range(8, 24)) + list(range(0, 8))
P3_KS = list(range(8, 16)) + list(range(0, 8))

# --- B chunk stream: (name, [(v, k), ...]) in exact consumption order ---
def _b_chunks():
    ch = []
    ch.append(("b3_a1", [(3, k) for k in range(8, 10)]))
    ch.append(("b3_a2", [(3, k) for k in range(10, 12)]))
    ch.append(("b3_b", [(3, k) for k in range(12, 16)]))
    ch.append(("b3_c", [(3, k) for k in range(16, 20)]))
    ch.append(("b3_d", [(3, k) for k in range(20, 24)]))
    ch.append(("b3_e", [(3, k) for k in range(24, 28)]))
    ch.append(("b3_f", [(3, k) for k in range(28, 32)]))
    ch.append(("b3_lo", [(3, k) for k in range(0, 8)]))
    ch.append(("b2_a", [(2, k) for k in range(8, 16)]))
    ch.append(("b2_b", [(2, k) for k in range(16, 24)]))
    blo = []
    for k in range(8):
        blo.append((2, k))
        blo.append((0, k))
    ch.append(("b20_lo", blo))
    ch.append(("b1_a", [(1, k) for k in range(8, 16)]))
    ch.append(("b1_lo", [(1, k) for k in range(0, 8)]))
    return ch


B_CHUNKS = _b_chunks()

# chunk column layout within the single B DRAM tensor
B_CHUNK_BASE = {}
B_BLOCK_OFF = {}  # (v,k) -> (chunk_name, col offset within chunk, width)
_b = 0
for _name, _blocks in B_CHUNKS:
    B_CHUNK_BASE[_name] = _b
    _o = 0
    for _v, _k in _blocks:
        B_BLOCK_OFF[(_v, _k)] = (_name, _o, _wB(_k, _v))
        _o += _wB(_k, _v)
    _b += _o
B_COLS = _b  # 33792

# DMA issue order on the sync queue (A chunks interleaved at first need)
STREAM = [
    ("A", "aq2a"),
    ("B", "b3_a1"),
    ("A", "aq2b"),
    ("B", "b3_a2"),
    ("A", "aq3"),
    ("B", "b3_b"),
    ("A", "aq4"),
    ("B", "b3_c"),
    ("A", "aq5"),
    ("B", "b3_d"),
    ("A", "aq6"),
    ("B", "b3_e"),
    ("A", "aq7"),
    ("B", "b3_f"),
    ("A", "aq01"),
    ("B", "b3_lo"),
    ("B", "b2_a"),
    ("B", "b2_b"),
    ("B", "b20_lo"),
    ("B", "b1_a"),
    ("B", "b1_lo"),
]

# --- C layout: blocks in drain-batch order ---
CBLOCKS = (
    [(3, t) for t in range(2, 8)]
    + [(3, 0), (3, 1)]
    + [(2, 0), (2, 1), (0, 0), (0, 1)]
    + [(2, t) for t in range(2, 6)]
    + [(1, 2), (1, 3)]
    + [(1, 0), (1, 1)]
)
CBASE = {}
_c = 0
for _v, _t in CBLOCKS:
    CBASE[(_v, _t)] = _c
    _c += _wC(_v, _t)
C_COLS = _c  # 9216

# drain batches: (name, [(v, t)...], n_store_dmas)
DRAIN_BATCHES = [
    ("c3hi", [(3, t) for t in range(2, 8)], 2),
    ("c3lo", [(3, 0), (3, 1)], 1),
    ("c20lo", [(2, 0), (2, 1), (0, 0), (0, 1)], 1),
    ("c2hi", [(2, t) for t in range(2, 6)], 1),
    ("c1hi", [(1, 2), (1, 3)], 1),
    ("c1lo", [(1, 0), (1, 1)], 1),
]

_cache = {}


def _build():
    import concourse.bacc as bacc
    import concourse.mybir as mybir
    import concourse.tile as tile

    D = mybir.dt.bfloat16
    F32 = mybir.dt.float32
    Copy = mybir.ActivationFunctionType.Copy

    nc = bacc.Bacc(None, target_bir_lowering=False)
    AT = nc.dram_tensor("AT", [P, A_COLS], D, kind="ExternalInput")
    Bm = nc.dram_tensor("B", [P, B_COLS], D, kind="ExternalInput")
    Cm = nc.dram_tensor("C", [P, C_COLS], D, kind="ExternalOutput")

    with tile.TileContext(nc) as tc:
        with (
            tc.tile_pool(name="w", bufs=1) as wpool,
            tc.tile_pool(name="a", bufs=1) as apool,
            tc.tile_pool(name="b", bufs=1) as bpool,
            tc.tile_pool(name="o", bufs=1) as opool,
            tc.tile_pool(name="ps", bufs=8, space="PSUM") as pspool,
        ):
            # --- PE warmup during preamble / first DMA window ---
            warm = wpool.tile([P, SW], D, tag="wm", name="warm")
            nc.vector.memset(warm[:], 0)
            wps = pspool.tile([P, SW], F32, tag="ps", name="ps")
            for _ in range(N_WARM):
                nc.tensor.matmul(
                    wps[:], warm[:, :P], warm[:], start=True, stop=True
                )

            # --- eager input DMAs, one queue, exact need order ---
            a_tiles = {}
            b_tiles = {}
            b_chunk_map = dict(B_CHUNKS)
            for kind, name in STREAM:
                if kind == "A":
                    c0, c1 = A_CHUNKS[name]
                    at = apool.tile([P, c1 - c0], D, tag=name, name="at")
                    nc.sync.dma_start(at[:], AT[:, c0:c1])
                    a_tiles[name] = (at, c0)
                else:
                    base = B_CHUNK_BASE[name]
                    w = sum(_wB(k, v) for v, k in b_chunk_map[name])
                    bt = bpool.tile([P, w], D, tag=name, name="bt")
                    nc.sync.dma_start(bt[:], Bm[:, base : base + w])
                    b_tiles[name] = bt

            def a_sl(k, t):
                at, c0 = a_tiles[_a_chunk_name(k)]
                off = _aoff(k) - c0 + 128 * t
                return at[:, off : off + 128]

            def b_sl(v, k):
                name, off, w = B_BLOCK_OFF[(v, k)]
                return b_tiles[name][:, off : off + w]

            # --- psum allocation helper (ring order is load-bearing) ---
            def alloc_ps():
                return pspool.tile([P, SW], F32, tag="ps", name="ps")

            def mm(ps, v, k, t, first_k, last_k):
                w0 = 64 * _j0(k, v)
                nc.tensor.matmul(
                    ps[:, w0:SW],
                    a_sl(k, t),
                    b_sl(v, k),
                    start=(k == first_k),
                    stop=(k == last_k),
                )

            drain_eng = [0]  # alternate vector/scalar

            def drain_and_store(batch_name, blocks, psums, n_dmas):
                cols = sum(_wC(v, t) for v, t in blocks)
                ot = opool.tile([P, cols], D, tag=batch_name, name="ot")
                l0 = 0
                for v, t in blocks:
                    w0 = 64 * _jc0(v, t)
                    cw = _wC(v, t)
                    src = psums[(v, t)][:, w0:SW]
                    if drain_eng[0] % 2 == 0:
                        nc.vector.tensor_copy(ot[:, l0 : l0 + cw], src)
                    else:
                        nc.scalar.activation(ot[:, l0 : l0 + cw], src, Copy)
                    drain_eng[0] += 1
                    l0 += cw
                base = CBASE[blocks[0]]
                splits = [cols * i // n_dmas for i in range(n_dmas + 1)]
                for s0, s1 in zip(splits, splits[1:]):
                    nc.gpsimd.dma_start(
                        Cm[:, base + s0 : base + s1], ot[:, s0:s1]
                    )

            # ============ P1: v3, k = [8..31, 0..7] ============
            ps3 = {}
            for t in range(8):
                ps3[(3, t)] = alloc_ps()
            for k in P1_KS:
                for t in range(min(k // 4, 7) + 1):
                    first_k = 4 * t if 4 * t >= 8 else 8
                    last_k = 7 if t <= 1 else 31
                    mm(ps3[(3, t)], 3, k, t, first_k, last_k)
                if k == 31:  # t2..t7 complete: drain during the lo block
                    drain_and_store("c3hi", [(3, t) for t in range(2, 8)], ps3, 2)
            drain_and_store("c3lo", [(3, 0), (3, 1)], ps3, 1)

            # ============ P2: v2 + v0, k = [8..23, 0..7] ============
            ps2 = {}
            # alloc order binds ring bufs: first-needed psums get
            # earliest-freed banks (v3 t0/t1 freed mid-P1, t2.. at k31).
            ps2[(2, 0)] = alloc_ps()  # buf of v3 t0 (freed mid-P1)
            ps2[(2, 1)] = alloc_ps()  # v3 t1 (freed mid-P1)
            ps2[(2, 2)] = alloc_ps()  # v3 t2 (freed at P1 end)
            ps2[(2, 3)] = alloc_ps()
            ps2[(2, 4)] = alloc_ps()
            ps2[(2, 5)] = alloc_ps()
            ps2[(0, 0)] = alloc_ps()
            ps2[(0, 1)] = alloc_ps()
            for k in P2_KS:
                ts = list(range(min(k // 4, 5) + 1))
                if k == 8:
                    ts = [2, 0, 1]  # t2's psum bank frees first (P1 hi end)
                for t in ts:
                    first_k = 4 * t if 4 * t >= 8 else 8
                    last_k = 7 if t <= 1 else 23
                    mm(ps2[(2, t)], 2, k, t, first_k, last_k)
                if k < 8:  # lo block: v0 runs alongside
                    for t in range(min(k // 4, 1) + 1):
                        mm(ps2[(0, t)], 0, k, t, 4 * t, 7)
                if k == 23:  # t2..t5 complete: drain during the lo block
                    drain_and_store("c2hi", [(2, t) for t in range(2, 6)], ps2, 1)
            drain_and_store("c20lo", [(2, 0), (2, 1), (0, 0), (0, 1)], ps2, 1)

            # ============ P3: v1, k = [8..15, 0..7] ============
            ps1 = {}
            for t in range(4):
                ps1[(1, t)] = alloc_ps()
            for k in P3_KS:
                ts = list(range(min(k // 4, 3) + 1))
                if k == 8:
                    ts = [2, 0, 1]  # t2's psum bank frees first (P2 hi end)
                elif k == 7:
                    ts = [1, 0]  # t1 stops first so its drain starts earlier
                for t in ts:
                    first_k = 4 * t if 4 * t >= 8 else 8
                    last_k = 7 if t <= 1 else 15
                    mm(ps1[(1, t)], 1, k, t, first_k, last_k)
                if k == 15:  # t2, t3 complete: drain during the lo block
                    drain_and_store("c1hi", [(1, 2), (1, 3)], ps1, 1)
            # tail: per-slot copy+store on separate queues, each store gated
            # on only its own copy
            ot1 = opool.tile([P, _wC(1, 1)], D, tag="c1t1", name="ot1")
            nc.scalar.activation(ot1[:], ps1[(1, 1)][:, 0:SW], Copy)
            nc.sync.dma_start(
                Cm[:, CBASE[(1, 1)] : CBASE[(1, 1)] + _wC(1, 1)], ot1[:]
            )
            ot0 = opool.tile([P, _wC(1, 0)], D, tag="c1t0", name="ot0")
            nc.vector.tensor_copy(ot0[:], ps1[(1, 0)][:, 0:SW])
            nc.gpsimd.dma_start(
                Cm[:, CBASE[(1, 0)] : CBASE[(1, 0)] + _wC(1, 0)], ot0[:]
            )

    nc.compile()
    return nc


def _get_nc():
    if "nc" not in _cache:
        _cache["nc"] = _build()
    return _cache["nc"]


def _make_in_maps(A, B):
    import ml_dtypes

    bf16 = np.dtype(ml_dtypes.bfloat16)
    A = np.asarray(A, dtype=np.float32)
    B = np.asarray(B, dtype=np.float32)
    Au = np.triu(A).astype(bf16)
    Bu = np.triu(B).astype(bf16)

    # A packs per row-group r: [p, k-major slots], lhsT layout
    a_packs = []
    for r in range(4):
        ATr = np.zeros((P, A_COLS), dtype=bf16)
        for k in range(KT):
            base = _aoff(k)
            for t in range(k // 4 + 1):
                m = 4 * t + r
                ATr[:, base + 128 * t : base + 128 * (t + 1)] = Au[
                    128 * m : 128 * m + 128, 128 * k : 128 * k + 128
                ].T
        a_packs.append(ATr)

    # B packs per column parity h: chunk-stream layout, 64-wide strips
    b_packs = []
    for h in range(2):
        Bh = np.zeros((P, B_COLS), dtype=bf16)
        for name, blocks in B_CHUNKS:
            base = B_CHUNK_BASE[name]
            for v, k in blocks:
                _, off, w = B_BLOCK_OFF[(v, k)]
                for i, j in enumerate(range(_j0(k, v), 8)):
                    n64 = 16 * v + 2 * j + h
                    Bh[:, base + off + 64 * i : base + off + 64 * (i + 1)] = (
                        Bu[128 * k : 128 * k + 128, 64 * n64 : 64 * n64 + 64]
                    )
        b_packs.append(Bh)

    in_maps = []
    for jcore in range(NCORES):
        r, h = jcore % 4, jcore // 4
        in_maps.append({"AT": a_packs[r], "B": b_packs[h]})
    return in_maps


def kernel(A, B):
    from concourse.bass_utils import run_bass_kernel_spmd

    in_maps = _make_in_maps(A, B)
    nc = _get_nc()
    res = run_bass_kernel_spmd(nc, in_maps, core_ids=list(range(NCORES)))

    C = np.zeros((N, N), dtype=np.float32)
    for jcore in range(NCORES):
        r, h = jcore % 4, jcore // 4
        Cj = np.asarray(res.results[jcore]["C"]).astype(np.float32)
        for v, t in CBLOCKS:
            m = 4 * t + r
            cb = CBASE[(v, t)]
            for i, j in enumerate(range(_jc0(v, t), 8)):
                n64 = 16 * v + 2 * j + h
                C[128 * m : 128 * m + 128, 64 * n64 : 64 * n64 + 64] = Cj[
                    :, cb + 64 * i : cb + 64 * (i + 1)
                ]
    return C
